# revision 4
# baseline (speedup 1.0000x reference)
"""DeepReservoir (2-layer leaky ESN, T=8192, units=1024) on 8 trn2 cores.

Strategy: parallel-in-time with washout. Each core owns a contiguous
1024-step span of the sequence, split into B=64 chunks of L=16 steps.
The chunks advance in lockstep as the free dimension of the recurrent
matmuls (batched matvec), each chunk cold-started from h=0 with a
washout warmup (the ESN's fading memory makes the state converge to the
true trajectory at ~0.77/step; W=128 warmup steps reach the fp32 noise
floor). Core 0's first chunk is exact: its padded input projections are
zero, which keeps h pinned at exactly 0 through the warmup.

State is tracked as s=2h so the leaky blend is one fused DVE op
(s' = 0.5*s + tanh(z)); the 0.5 factors are folded into host-prescaled
weights. Biases are folded into the projection matmuls via an
augmented ones-row whose padding zeros also kill the bias on
out-of-range rows.
"""

import numpy as np

import concourse.bass as bass
import concourse.mybir as mybir
from concourse import bacc
from concourse.bass import ds
from concourse.tile import TileContext
from concourse.bass_utils import run_bass_kernel_spmd

# problem constants
T = 8192
UNITS = 1024
IN = 32
NCORES = 8
LEAKY = 0.5
P = 128
NCH = UNITS // P  # 8 unit chunks

# tuning
W_WASH = 128          # washout steps
B = 64                # time chunks per core (matmul free dim)
SPAN = T // NCORES    # 1024 steps per core
L = SPAN // B         # 16 steps per chunk
S0 = 2 * W_WASH + L   # module-0 scan steps (272)
S1 = W_WASH + L       # module-1 scan steps (144)
X0C = SPAN + 2 * W_WASH   # X0 columns (1280)
X1C = SPAN + W_WASH       # X1 columns (1152)
HB = X0C + 1              # hbuf columns (1281)
UNROLL = 8            # For_i unroll (must be even for ping-pong parity)

FP = mybir.dt.float32
AF = mybir.ActivationFunctionType
OP = mybir.AluOpType

_CACHE = {}


def _build():
    nc = bacc.Bacc()
    d_w0 = nc.dram_tensor("w0h", [UNITS, UNITS], FP, kind="ExternalInput")
    d_w1 = nc.dram_tensor("w1h", [UNITS, UNITS], FP, kind="ExternalInput")
    d_k1 = nc.dram_tensor("k1h", [UNITS, UNITS], FP, kind="ExternalInput")
    d_k0 = nc.dram_tensor("k0aug", [IN + 1, UNITS], FP, kind="ExternalInput")
    d_b1 = nc.dram_tensor("b1row", [1, UNITS], FP, kind="ExternalInput")
    d_u = nc.dram_tensor("u_aug", [IN + 1, X0C], FP, kind="ExternalInput")
    d_on = nc.dram_tensor("ones1", [1, X1C], FP, kind="ExternalInput")
    d_out = nc.dram_tensor("out", [SPAN, 2 * UNITS], FP, kind="ExternalOutput")

    def out_dma(col_block, out_i, src):
        # per unit-chunk: dst dims [p: step 1][t: step 2*UNITS*L, count B]
        for c in range(NCH):
            nc.sync.dma_start(
                out=bass.AP(
                    d_out,
                    out_i * (2 * UNITS) + col_block * UNITS + c * P,
                    [[1, P], [2 * UNITS * L, B]]),
                in_=src[:, c, :])

    with TileContext(nc) as tc:
        with tc.tile_pool(name="sb", bufs=1) as pool, \
             tc.tile_pool(name="ps", bufs=1, space="PSUM") as psp:
            wbuf = pool.tile([P, NCH, UNITS], FP)    # W0h blocks; later W1h
            k1buf = pool.tile([P, NCH, UNITS], FP)   # K1h blocks
            k0buf = pool.tile([IN + 1, UNITS], FP)
            b1buf = pool.tile([1, UNITS], FP)
            uin = pool.tile([IN + 1, X0C], FP)
            ones1 = pool.tile([1, X1C], FP)
            xbuf = pool.tile([P, NCH, X0C], FP)      # X0x, then X1x
            hbuf = pool.tile([P, NCH, HB], FP)       # s0 trajectory
            h0p = [pool.tile([P, NCH, B], FP, name=f"h0p{i}", tag=f"h0p{i}")
                   for i in range(2)]
            h1p = [pool.tile([P, NCH, B], FP, name=f"h1p{i}", tag=f"h1p{i}")
                   for i in range(2)]
            zg = pool.tile([P, NCH, B], FP)
            gt = pool.tile([P, NCH, B], FP)
            hout = pool.tile([P, NCH, B], FP)
            ps_s = psp.tile([P, NCH * B], FP)        # scan psum (1 bank)
            ps_x = psp.tile([P, 1536], FP)           # projection psum (3 banks)

            # ---- preamble loads ----
            for c in range(NCH):
                nc.sync.dma_start(out=wbuf[:, c, :], in_=d_w0[c * P:(c + 1) * P, :])
                nc.sync.dma_start(out=k1buf[:, c, :], in_=d_k1[c * P:(c + 1) * P, :])
            nc.sync.dma_start(out=k0buf[:], in_=d_k0[:])
            nc.sync.dma_start(out=b1buf[:], in_=d_b1[:])
            nc.sync.dma_start(out=uin[:], in_=d_u[:])
            nc.sync.dma_start(out=ones1[:], in_=d_on[:])
            nc.vector.memset(h0p[0][:], 0.0)
            nc.vector.memset(h1p[0][:], 0.0)

            # ---- P0: X0x = K0aug.T @ u_aug  -> xbuf ----
            nt_list = [(0, 512), (512, 512), (1024, X0C - 1024)]
            for d in range(NCH):
                for (o, n) in nt_list:
                    nc.tensor.matmul(
                        ps_x[:, o:o + n],
                        k0buf[:, d * P:(d + 1) * P],
                        uin[:, o:o + n],
                        start=True, stop=True)
                nc.scalar.activation(xbuf[:, d, :], ps_x[:, 0:X0C], AF.Copy)

            # ---- P1: module-0 scan ----
            def step0(i, par, out_i=None):
                """one scan step; i: element offset (ScalarValue or int),
                par: ping-pong parity (python int), out_i: python int step
                index when in the output phase."""
                hi, ho = h0p[par], h0p[1 - par]
                for d in range(NCH):
                    for c in range(NCH):
                        nc.tensor.matmul(
                            ps_s[:, d * B:(d + 1) * B],
                            wbuf[:, c, d * P:(d + 1) * P],
                            hi[:, c, :],
                            start=(c == 0), stop=(c == NCH - 1))
                for d in range(NCH):
                    nc.vector.tensor_tensor(
                        out=zg[:, d, :],
                        in0=ps_s[:, d * B:(d + 1) * B],
                        in1=xbuf[:, d, ds(i, B, L)],
                        op=OP.add)
                    nc.scalar.activation(gt[:, d, :], zg[:, d, :], AF.Tanh)
                    nc.vector.scalar_tensor_tensor(
                        out=ho[:, d, :], in0=hi[:, d, :], scalar=0.5,
                        in1=gt[:, d, :], op0=OP.mult, op1=OP.add)
                # trajectory write (s values)
                nc.vector.tensor_copy(out=hbuf[:, :, ds(i + 1, B, L)], in_=ho[:])
                if out_i is not None:
                    nc.vector.tensor_scalar_mul(hout[:], ho[:], 0.5)
                    out_dma(0, out_i, hout)

            tc.For_i_unrolled_general(
                0, 2 * W_WASH, 1,
                lambda iv, unroll: [step0(iv + j, j % 2) for j in range(unroll)],
                max_unroll=UNROLL)
            for i in range(2 * W_WASH, S0):
                step0(i, i % 2, out_i=i - 2 * W_WASH)

            # ---- load W1 into wbuf (after P1's last use) ----
            for c in range(NCH):
                nc.sync.dma_start(out=wbuf[:, c, :], in_=d_w1[c * P:(c + 1) * P, :])

            # ---- P2: X1x = K1h.T @ s0 + b1 (ones row) -> xbuf ----
            xt_list = [(0, 512), (512, 512), (1024, X1C - 1024)]
            for d in range(NCH):
                for c in range(NCH):
                    for (o, n) in xt_list:
                        nc.tensor.matmul(
                            ps_x[:, o:o + n],
                            k1buf[:, c, d * P:(d + 1) * P],
                            hbuf[:, c, W_WASH + 1 + o: W_WASH + 1 + o + n],
                            start=(c == 0), stop=False)
                for (o, n) in xt_list:
                    nc.tensor.matmul(
                        ps_x[:, o:o + n],
                        b1buf[:, d * P:(d + 1) * P],
                        ones1[:, o:o + n],
                        start=False, stop=True)
                nc.scalar.activation(xbuf[:, d, 0:X1C], ps_x[:, 0:X1C], AF.Copy)

            # ---- P3: module-1 scan ----
            def step1(j, par, out_j=None):
                hi, ho = h1p[par], h1p[1 - par]
                for d in range(NCH):
                    for c in range(NCH):
                        nc.tensor.matmul(
                            ps_s[:, d * B:(d + 1) * B],
                            wbuf[:, c, d * P:(d + 1) * P],
                            hi[:, c, :],
                            start=(c == 0), stop=(c == NCH - 1))
                for d in range(NCH):
                    nc.vector.tensor_tensor(
                        out=zg[:, d, :],
                        in0=ps_s[:, d * B:(d + 1) * B],
                        in1=xbuf[:, d, ds(j, B, L)],
                        op=OP.add)
                    nc.scalar.activation(gt[:, d, :], zg[:, d, :], AF.Tanh)
                    nc.vector.scalar_tensor_tensor(
                        out=ho[:, d, :], in0=hi[:, d, :], scalar=0.5,
                        in1=gt[:, d, :], op0=OP.mult, op1=OP.add)
                if out_j is not None:
                    nc.vector.tensor_scalar_mul(hout[:], ho[:], 0.5)
                    out_dma(1, out_j, hout)

            tc.For_i_unrolled_general(
                0, W_WASH, 1,
                lambda iv, unroll: [step1(iv + j, j % 2) for j in range(unroll)],
                max_unroll=UNROLL)
            for j in range(W_WASH, S1):
                step1(j, j % 2, out_j=j - W_WASH)

    nc.compile()
    return nc


def _host_inputs(u, kernel0, rec0, bias0, kernel1, rec1, bias1):
    u = np.asarray(u, dtype=np.float32).reshape(T, IN)
    w0h = (0.5 * np.asarray(rec0, dtype=np.float32)).copy()
    w1h = (0.5 * np.asarray(rec1, dtype=np.float32)).copy()
    k1h = (0.5 * np.asarray(kernel1, dtype=np.float32)).copy()
    k0aug = np.concatenate(
        [np.asarray(kernel0, dtype=np.float32),
         np.asarray(bias0, dtype=np.float32).reshape(1, UNITS)], axis=0)
    b1row = np.asarray(bias1, dtype=np.float32).reshape(1, UNITS).copy()

    in_maps = []
    for core in range(NCORES):
        s0 = core * SPAN
        lo = s0 - 2 * W_WASH
        u_aug = np.zeros((IN + 1, X0C), dtype=np.float32)
        for q in range(X0C):
            t = lo + q
            if t >= 0:
                u_aug[:IN, q] = u[t]
                u_aug[IN, q] = 1.0
        ones1 = np.zeros((1, X1C), dtype=np.float32)
        lo1 = s0 - W_WASH
        for r in range(X1C):
            if lo1 + r >= 0:
                ones1[0, r] = 1.0
        in_maps.append({
            "w0h": w0h, "w1h": w1h, "k1h": k1h, "k0aug": k0aug,
            "b1row": b1row, "u_aug": u_aug, "ones1": ones1,
        })
    return in_maps


def kernel(u, kernel0, rec0, bias0, kernel1, rec1, bias1):
    if "nc" not in _CACHE:
        _CACHE["nc"] = _build()
    nc = _CACHE["nc"]
    in_maps = _host_inputs(u, kernel0, rec0, bias0, kernel1, rec1, bias1)
    res = run_bass_kernel_spmd(nc, in_maps, core_ids=list(range(NCORES)))
    out = np.concatenate([res.results[c]["out"] for c in range(NCORES)], axis=0)
    return out.reshape(1, T, 2 * UNITS).astype(np.float32)


# revision 5
# speedup vs baseline: 2.8985x; 2.8985x over previous
"""DeepReservoir (2-layer leaky ESN, T=8192, units=1024) on 8 trn2 cores.

Strategy: parallel-in-time with washout. Each core owns a contiguous
1024-step span, split into B=128 chunks of L=8 steps. The chunks
advance in lockstep as the free dimension of the recurrent matmuls
(batched matvec), each chunk cold-started from h=0 with a washout
warmup (fading memory converges at ~0.8/step; W=96 warmup steps put
the truncation error well under the arithmetic noise floor). Core 0's
first chunk is exact: its padded input projections are zero, which
keeps h pinned at exactly 0 through the warmup.

The 1024x1024 recurrent matmul runs as bf16x2 split-precision
(W ~ W_hi + W_lo, s ~ s_hi + s_lo; z ~ s_hi@W_hi + s_lo@W_hi +
s_hi@W_lo with fp32 PSUM accumulation, ~2^-17 relative error) because
fp32 matmuls cost 2 LDWEIGHTS+2 MATMUL passes on trn2 while bf16 loads
stream at full rate. The master state s stays fp32.

State is tracked as s=2h so the leaky blend is one fused DVE op
(s' = 0.5*s + tanh(z)); the 0.5 factors are folded into host-prescaled
weights. Biases are folded into the projection matmuls via an
augmented ones-row whose padding zeros also kill the bias on
out-of-range rows. Outputs are written to DRAM in the on-chip layout
and reordered on the host.
"""

import numpy as np

import concourse.bass as bass
import concourse.mybir as mybir
from concourse import bacc
from concourse.bass import ds
from concourse.tile import TileContext
from concourse.bass_utils import run_bass_kernel_spmd

# problem constants
T = 8192
UNITS = 1024
IN = 32
NCORES = 8
P = 128
NCH = UNITS // P  # 8 unit chunks

# tuning
W_WASH = 96           # washout steps
B = 128               # time chunks per core (matmul free dim)
SPAN = T // NCORES    # 1024 steps per core
L = SPAN // B         # 8 steps per chunk
S0 = 2 * W_WASH + L   # module-0 scan steps (200)
S1 = W_WASH + L       # module-1 scan steps (104)
X0C = SPAN + 2 * W_WASH   # X0 columns (1216)
X1C = SPAN + W_WASH       # X1 columns (1120)
HB = X0C + 1              # hbuf columns (1217)
UNROLL = 8            # For_i unroll (must be even for ping-pong parity)

FP = mybir.dt.float32
BF = mybir.dt.bfloat16
AF = mybir.ActivationFunctionType
OP = mybir.AluOpType

_CACHE = {}


def _build():
    nc = bacc.Bacc()
    dw = {}
    for nm in ["w0hi", "w0lo", "w1hi", "w1lo", "k1hi", "k1lo"]:
        dw[nm] = nc.dram_tensor(nm, [UNITS, UNITS], BF, kind="ExternalInput")
    d_k0 = nc.dram_tensor("k0aug", [IN + 1, UNITS], FP, kind="ExternalInput")
    d_b1 = nc.dram_tensor("b1row", [1, UNITS], FP, kind="ExternalInput")
    d_u = nc.dram_tensor("u_aug", [IN + 1, X0C], FP, kind="ExternalInput")
    d_on = nc.dram_tensor("ones1", [1, X1C], FP, kind="ExternalInput")
    d_out0 = nc.dram_tensor("out0", [L, P, NCH * B], FP, kind="ExternalOutput")
    d_out1 = nc.dram_tensor("out1", [L, P, NCH * B], FP, kind="ExternalOutput")

    with TileContext(nc) as tc:
        with tc.tile_pool(name="sb", bufs=1) as pool, \
             tc.tile_pool(name="ps", bufs=1, space="PSUM") as psp:
            whi = pool.tile([P, NCH, UNITS], BF)   # W0hi, later W1hi
            wlo = pool.tile([P, NCH, UNITS], BF)   # W0lo, later W1lo
            k1hi = pool.tile([P, NCH, UNITS], BF)
            k1lo = pool.tile([P, NCH, UNITS], BF)
            k0buf = pool.tile([IN + 1, UNITS], FP)
            b1buf = pool.tile([1, UNITS], FP)
            uin = pool.tile([IN + 1, X0C], FP)
            ones1 = pool.tile([1, X1C], FP)
            xbuf = pool.tile([P, NCH, X0C], FP)    # X0x, then X1x
            hbhi = pool.tile([P, NCH, HB], BF)     # s0 trajectory (hi)
            hblo = pool.tile([P, NCH, HB], BF)     # s0 trajectory (lo)
            hm = [pool.tile([P, NCH, B], FP, name=f"hm{i}") for i in range(2)]
            shi = [pool.tile([P, NCH, B], BF, name=f"shi{i}") for i in range(2)]
            slo = [pool.tile([P, NCH, B], BF, name=f"slo{i}") for i in range(2)]
            zg = pool.tile([P, NCH, B], FP)
            gt = pool.tile([P, NCH, B], FP)
            hout = pool.tile([P, NCH, B], FP)
            ps_s = psp.tile([P, NCH * B], FP)      # scan psum (2 banks)
            ps_x = psp.tile([P, 1536], FP)         # projection psum (3 banks)

            # ---- preamble loads ----
            for c in range(NCH):
                nc.sync.dma_start(out=whi[:, c, :], in_=dw["w0hi"][c * P:(c + 1) * P, :])
                nc.sync.dma_start(out=wlo[:, c, :], in_=dw["w0lo"][c * P:(c + 1) * P, :])
                nc.sync.dma_start(out=k1hi[:, c, :], in_=dw["k1hi"][c * P:(c + 1) * P, :])
                nc.sync.dma_start(out=k1lo[:, c, :], in_=dw["k1lo"][c * P:(c + 1) * P, :])
            nc.sync.dma_start(out=k0buf[:], in_=d_k0[:])
            nc.sync.dma_start(out=b1buf[:], in_=d_b1[:])
            nc.sync.dma_start(out=uin[:], in_=d_u[:])
            nc.sync.dma_start(out=ones1[:], in_=d_on[:])
            for t in (hm[0], shi[0], slo[0]):
                nc.vector.memset(t[:], 0.0)

            # ---- P0: X0x = K0aug.T @ u_aug  -> xbuf (fp32) ----
            nt_list = [(0, 512), (512, 512), (1024, X0C - 1024)]
            for d in range(NCH):
                for (o, n) in nt_list:
                    nc.tensor.matmul(
                        ps_x[:, o:o + n],
                        k0buf[:, d * P:(d + 1) * P],
                        uin[:, o:o + n],
                        start=True, stop=True)
                nc.scalar.activation(xbuf[:, d, :], ps_x[:, 0:X0C], AF.Copy)

            # ---- scan step (shared for both modules) ----
            def step(i, par, mod, out_i=None):
                hi_m, ho_m = hm[par], hm[1 - par]
                hi_h, ho_h = shi[par], shi[1 - par]
                hi_l, ho_l = slo[par], slo[1 - par]
                for d in range(NCH):
                    psl = ps_s[:, d * B:(d + 1) * B]
                    for c in range(NCH):
                        wsl = (slice(None), c, slice(d * P, (d + 1) * P))
                        nc.tensor.matmul(psl, whi[wsl], hi_h[:, c, :],
                                         start=(c == 0), stop=False)
                        nc.tensor.matmul(psl, whi[wsl], hi_l[:, c, :],
                                         start=False, stop=False)
                        nc.tensor.matmul(psl, wlo[wsl], hi_h[:, c, :],
                                         start=False, stop=(c == NCH - 1))
                for d in range(NCH):
                    nc.vector.tensor_tensor(
                        out=zg[:, d, :],
                        in0=ps_s[:, d * B:(d + 1) * B],
                        in1=xbuf[:, d, ds(i, B, L)],
                        op=OP.add)
                    nc.scalar.activation(gt[:, d, :], zg[:, d, :], AF.Tanh)
                    nc.vector.scalar_tensor_tensor(
                        out=ho_m[:, d, :], in0=hi_m[:, d, :], scalar=0.5,
                        in1=gt[:, d, :], op0=OP.mult, op1=OP.add)
                # split next-state into bf16 hi/lo for the next matmuls
                nc.vector.tensor_copy(out=ho_h[:], in_=ho_m[:])
                nc.vector.tensor_tensor(out=ho_l[:], in0=ho_m[:], in1=ho_h[:],
                                        op=OP.subtract)
                if mod == 0:
                    # s0 trajectory for the X1 projection
                    nc.vector.tensor_copy(out=hbhi[:, :, ds(i + 1, B, L)], in_=ho_h[:])
                    nc.vector.tensor_copy(out=hblo[:, :, ds(i + 1, B, L)], in_=ho_l[:])
                if out_i is not None:
                    nc.vector.tensor_scalar_mul(hout[:], ho_m[:], 0.5)
                    dst = d_out0 if mod == 0 else d_out1
                    nc.sync.dma_start(out=dst[out_i], in_=hout[:])

            # ---- P1: module-0 scan ----
            tc.For_i_unrolled_general(
                0, 2 * W_WASH, 1,
                lambda iv, unroll: [step(iv + j, j % 2, 0) for j in range(unroll)],
                max_unroll=UNROLL)
            for i in range(2 * W_WASH, S0):
                step(i, i % 2, 0, out_i=i - 2 * W_WASH)

            # ---- load W1 into whi/wlo (after P1's last use) ----
            for c in range(NCH):
                nc.sync.dma_start(out=whi[:, c, :], in_=dw["w1hi"][c * P:(c + 1) * P, :])
                nc.sync.dma_start(out=wlo[:, c, :], in_=dw["w1lo"][c * P:(c + 1) * P, :])

            # ---- P2: X1x = K1h.T @ s0 + b1 (ones row) -> xbuf ----
            xt_list = [(0, 512), (512, 512), (1024, X1C - 1024)]
            for d in range(NCH):
                for c in range(NCH):
                    for (o, n) in xt_list:
                        hsl = slice(W_WASH + 1 + o, W_WASH + 1 + o + n)
                        psl = ps_x[:, o:o + n]
                        ksl = (slice(None), c, slice(d * P, (d + 1) * P))
                        nc.tensor.matmul(psl, k1hi[ksl], hbhi[:, c, hsl],
                                         start=(c == 0), stop=False)
                        nc.tensor.matmul(psl, k1hi[ksl], hblo[:, c, hsl],
                                         start=False, stop=False)
                        nc.tensor.matmul(psl, k1lo[ksl], hbhi[:, c, hsl],
                                         start=False, stop=False)
                for (o, n) in xt_list:
                    nc.tensor.matmul(
                        ps_x[:, o:o + n],
                        b1buf[:, d * P:(d + 1) * P],
                        ones1[:, o:o + n],
                        start=False, stop=True)
                nc.scalar.activation(xbuf[:, d, 0:X1C], ps_x[:, 0:X1C], AF.Copy)

            # reset scan state for module 1
            for t in (hm[0], shi[0], slo[0]):
                nc.vector.memset(t[:], 0.0)

            # ---- P3: module-1 scan ----
            tc.For_i_unrolled_general(
                0, W_WASH, 1,
                lambda iv, unroll: [step(iv + j, j % 2, 1) for j in range(unroll)],
                max_unroll=UNROLL)
            for j in range(W_WASH, S1):
                step(j, j % 2, 1, out_i=j - W_WASH)

    nc.compile()
    return nc


def _bf16_pair(x):
    import ml_dtypes
    hi = x.astype(ml_dtypes.bfloat16)
    lo = (x - hi.astype(np.float32)).astype(ml_dtypes.bfloat16)
    return hi, lo


def _host_inputs(u, kernel0, rec0, bias0, kernel1, rec1, bias1):
    u = np.asarray(u, dtype=np.float32).reshape(T, IN)
    w0hi, w0lo = _bf16_pair(0.5 * np.asarray(rec0, dtype=np.float32))
    w1hi, w1lo = _bf16_pair(0.5 * np.asarray(rec1, dtype=np.float32))
    k1hi, k1lo = _bf16_pair(0.5 * np.asarray(kernel1, dtype=np.float32))
    k0aug = np.concatenate(
        [np.asarray(kernel0, dtype=np.float32),
         np.asarray(bias0, dtype=np.float32).reshape(1, UNITS)], axis=0)
    b1row = np.asarray(bias1, dtype=np.float32).reshape(1, UNITS).copy()

    in_maps = []
    for core in range(NCORES):
        s0 = core * SPAN
        lo_t = s0 - 2 * W_WASH
        u_aug = np.zeros((IN + 1, X0C), dtype=np.float32)
        n_pad = max(0, -lo_t)
        u_aug[:IN, n_pad:] = u[lo_t + n_pad: s0 + SPAN].T
        u_aug[IN, n_pad:] = 1.0
        ones1 = np.zeros((1, X1C), dtype=np.float32)
        ones1[0, max(0, W_WASH - s0):] = 1.0
        in_maps.append({
            "w0hi": w0hi, "w0lo": w0lo, "w1hi": w1hi, "w1lo": w1lo,
            "k1hi": k1hi, "k1lo": k1lo, "k0aug": k0aug,
            "b1row": b1row, "u_aug": u_aug, "ones1": ones1,
        })
    return in_maps


def _reorder(arr):
    # arr [L, P, NCH*B] -> [SPAN, UNITS]; element (i, p, c*B+s) is
    # h at (row s*L+i, col c*P+p)
    a = arr.reshape(L, P, NCH, B)
    return a.transpose(3, 0, 2, 1).reshape(SPAN, UNITS)


def kernel(u, kernel0, rec0, bias0, kernel1, rec1, bias1):
    if "nc" not in _CACHE:
        _CACHE["nc"] = _build()
    nc = _CACHE["nc"]
    in_maps = _host_inputs(u, kernel0, rec0, bias0, kernel1, rec1, bias1)
    res = run_bass_kernel_spmd(nc, in_maps, core_ids=list(range(NCORES)))
    out = np.empty((T, 2 * UNITS), dtype=np.float32)
    for c in range(NCORES):
        out[c * SPAN:(c + 1) * SPAN, :UNITS] = _reorder(res.results[c]["out0"])
        out[c * SPAN:(c + 1) * SPAN, UNITS:] = _reorder(res.results[c]["out1"])
    return out.reshape(1, T, 2 * UNITS)
